# revision 8
# baseline (speedup 1.0000x reference)
"""CameraAwareMemory loss kernel for 8 Trainium2 NeuronCores.

Strategy: camera-sharding — core k owns ALL 4096 proxies of camera k
(P=32768, C=8).  Each core computes score = feat @ memT with bf16
matmuls and sims' = (feat + r*mem[prx]) @ memT with fp8(e4m3) DoubleRow
matmuls (one instruction per 512-col half: the 256-deep contraction
rides the two packed k-halves), then reduces over 1024-col double-bank
PSUM tiles:
  - camera sum of exp(score/TEMP - mhat): one fused exp+accumulate on
    the scalar engine per 1024-col tile (4 partial sums per row-tile;
    host adds them).  mhat is a host-computed per-row bias, identical on
    all cores, statistically pinned to the row max.
  - per-64-proxy-window max of sims' via one DVE windowed tensor_reduce
    per 1024-col tile, direct on PSUM (16 windows per tile; the window
    POSITION identifies the proxies, so no max_index pass is needed)
The host merges the 8 cores' partials into the exact loss: the union of
the top-J windows per row provably covers every proxy the reference's
top-k selections can touch (a window containing the k-th largest value
always ranks within the top-k windows by window-max), and the host
recomputes exact fp32 scores at the candidate proxies so no selection
decision depends on fp8/bf16 rounding (fp8 only perturbs WHICH windows
are expanded; margins JG/DELTA absorb its ~0.05 value noise).
"""

import sys

import numpy as np

sys.path.insert(0, "/opt/trn_rl_repo")

# ---- problem constants (hardcoded per spec) ----
P = 32768
D = 256
C = 8
B = 256
TEMP = 0.05
BG_KNN = 50
POSK = 3
BAL_W = 0.15
RATIO = (1.0 - BAL_W) / BAL_W        # 5.666...: sims' = score + RATIO*q  (same order as sims)
INV_TEMP = 1.0 / TEMP                # 20.0
NCORES = 8
PSH = P // NCORES                    # 4096 proxies per core (= one camera)
WIN = 64                             # proxies per candidate window
NWIN_CORE = PSH // WIN               # 64 windows per core
NT = 4                               # 1024-col tiles per row-tile
TW = PSH // NT                       # 1024 cols per tile
OBW = NWIN_CORE + NT                 # out cols per row-tile: 64 maxes + 4 partial sums

_CACHE = {}


def _build_bass():
    import concourse.bacc as bacc
    import concourse.mybir as mybir
    import concourse.tile as tile
    from contextlib import ExitStack

    f32 = mybir.dt.float32
    bf16 = mybir.dt.bfloat16
    f8 = mybir.dt.float8e4
    AF = mybir.ActivationFunctionType

    nc = bacc.Bacc("TRN2", target_bir_lowering=False, debug=False)

    # bf16 pack: [feat lhsT (256 cols) | memT shard (4096 cols)] per k-half
    PKB = 256 + PSH                       # 4352
    packb_d = nc.dram_tensor("packb", [D, PKB], bf16, kind="ExternalInput")
    # fp8 pack: [sims lhsT (256 cols) | memT shard (4096 cols)] per k-half
    pack8_d = nc.dram_tensor("pack8", [D, PKB], f8, kind="ExternalInput")
    nbias_d = nc.dram_tensor("nbias", [B, 1], f32, kind="ExternalInput")
    ob_d = nc.dram_tensor("ob", [B, OBW], f32, kind="ExternalOutput")

    with tile.TileContext(nc) as tc, ExitStack() as ctx:
        consts = ctx.enter_context(tc.tile_pool(name="consts", bufs=1))
        pqp = ctx.enter_context(tc.tile_pool(name="pq", bufs=2, space="PSUM"))
        psp = ctx.enter_context(tc.tile_pool(name="psc", bufs=2, space="PSUM"))
        small = ctx.enter_context(tc.tile_pool(name="small", bufs=2))
        outp = ctx.enter_context(tc.tile_pool(name="outp", bufs=2))

        packb_sb = consts.tile([128, 2 * PKB], bf16, tag="packb")
        pack8_sb = consts.tile([128, 2 * PKB], f8, tag="pack8")
        pb_src = packb_d.rearrange("(k p) c -> p k c", k=2)
        pb_dst = packb_sb.rearrange("p (k c) -> p k c", k=2)
        p8_src = pack8_d.rearrange("(k p) c -> p k c", k=2)
        p8_dst = pack8_sb.rearrange("p (k c) -> p k c", k=2)

        # DMA order = first-use order: fp8 leads bf16 for every column range
        # (the in-order PE queue issues the fp8 sims matmul of a tile before
        # its bf16 score matmuls).  The first tile's operands ship as small
        # slivers so its activation can start ~1.7us earlier.
        nc.sync.dma_start(out=p8_dst[:, :, 0:768], in_=p8_src[:, :, 0:768])
        nc.sync.dma_start(out=pb_dst[:, 0, 0:768], in_=pb_src[:, 0, 0:768])
        nc.sync.dma_start(out=pb_dst[:, 1, 0:768], in_=pb_src[:, 1, 0:768])
        nc.sync.dma_start(out=p8_dst[:, :, 768:1280], in_=p8_src[:, :, 768:1280])
        nc.sync.dma_start(out=pb_dst[:, :, 768:1280], in_=pb_src[:, :, 768:1280])
        bbounds = [1280, 2304, 3328, PKB]
        for g in range(len(bbounds) - 1):
            lo, hi = bbounds[g], bbounds[g + 1]
            nc.sync.dma_start(out=p8_dst[:, :, lo:hi], in_=p8_src[:, :, lo:hi])
            nc.sync.dma_start(out=pb_dst[:, :, lo:hi], in_=pb_src[:, :, lo:hi])

        # warm scratch init on the otherwise-idle gpsimd queue, BEFORE the
        # nbias DMAs so the PE warm-up isn't blocked behind them.  The warm
        # matmuls keep the PE busy from ~1us so the clock is ramped when the
        # real matmuls start; a dummy activation preloads the Exp table.
        warm_in = consts.tile([128, 640], bf16, tag="warm")
        warm_f = consts.tile([128, 8], f32, tag="warmf")
        nc.gpsimd.memset(warm_f[:], 0.0)
        nc.gpsimd.memset(warm_in[:], 0.0)
        nc.scalar.activation(warm_f[:], warm_f[:], AF.Exp)

        # per-row-tile -mhat bias (tiny; idle gpsimd queue, needed later)
        mbneg = []
        for rt in range(2):
            t = small.tile([128, 1], f32, tag=f"mbneg{rt}", name=f"mbneg_{rt}")
            nc.gpsimd.dma_start(out=t[:], in_=nbias_d[rt * 128:(rt + 1) * 128, :])
            mbneg.append(t)

        wl, wr = warm_in[:, 0:128], warm_in[:, 128:640]
        warm_ps = pqp.tile([128, 512], f32, tag="ps", name="warm_ps")
        for _ in range(4):
            nc.tensor.matmul(warm_ps[:], lhsT=wl, rhs=wr, start=True, stop=True)

        ob_t = []
        for rt in range(2):
            ob_t.append(outp.tile([128, OBW], f32, tag=f"ob{rt}", name=f"ob_{rt}"))

        packb_3d = packb_sb.rearrange("p (k c) -> p k c", k=2)
        pack8_3d = pack8_sb.rearrange("p (k c) -> p k c", k=2)

        def wb_ap(k, rt):                      # bf16 feat lhsT [128, 128]
            return packb_3d[:, k, rt * 128:(rt + 1) * 128]

        def w8_ap(rt):                         # fp8 sims lhsT [128, 2, 128]
            return pack8_3d[:, :, rt * 128:(rt + 1) * 128]

        def rhsb(k, lo, hi):                   # bf16 memT cols [128, w]
            return packb_3d[:, k, 256 + lo: 256 + hi]

        def rhs8(lo, hi):                      # fp8 memT cols [128, 2, w]
            return pack8_3d[:, :, 256 + lo: 256 + hi]

        DR = mybir.MatmulPerfMode.DoubleRow

        for t in range(NT):
            lo = t * TW
            for rt in range(2):  # row-tile: batch rows [rt*128, rt*128+128)
                ps_q = pqp.tile([128, TW], f32, tag="ps", name=f"psq_{t}_{rt}")
                # fp8 DoubleRow: both k-halves in one pass, 512 cols per
                # instruction (fp8 moving-operand limit)
                for h in range(2):
                    nc.tensor.matmul(
                        ps_q[:, h * 512:(h + 1) * 512],
                        lhsT=w8_ap(rt), rhs=rhs8(lo + h * 512, lo + (h + 1) * 512),
                        start=True, stop=True, perf_mode=DR,
                    )
                # window maxes of sims': one DVE scan over both banks
                nc.vector.tensor_reduce(
                    out=ob_t[rt][:, t * (TW // WIN): (t + 1) * (TW // WIN)],
                    in_=ps_q[:].rearrange("p (w c) -> p w c", c=WIN),
                    axis=mybir.AxisListType.X,
                    op=mybir.AluOpType.max,
                )
                # score matmuls (bf16, 512-col PSUM-bank halves) + one fused
                # exp/accumulate over the whole 1024-col tile
                ps_s = psp.tile([128, TW], f32, tag="ps", name=f"pss_{t}_{rt}")
                for h in range(2):
                    for k in range(2):
                        nc.tensor.matmul(
                            ps_s[:, h * 512:(h + 1) * 512],
                            lhsT=wb_ap(k, rt),
                            rhs=rhsb(k, lo + h * 512, lo + (h + 1) * 512),
                            start=(k == 0), stop=(k == 1),
                        )
                # in-place exp: PSUM out avoids the slower SBUF write port and
                # an SBUF scratch tile (only the accum_out column is consumed)
                nc.scalar.activation(
                    ps_s[:], ps_s[:], AF.Exp,
                    bias=mbneg[rt][:], scale=INV_TEMP,
                    accum_out=ob_t[rt][:, NWIN_CORE + t: NWIN_CORE + t + 1],
                )

        for rt in range(2):
            r0, r1 = rt * 128, (rt + 1) * 128
            nc.sync.dma_start(out=ob_d[r0:r1, :], in_=ob_t[rt][:])

    nc.compile()
    return nc


def _get_nc():
    if "nc" not in _CACHE:
        _CACHE["nc"] = _build_bass()
    return _CACHE["nc"]


def _run_device(in_maps, trace=False):
    from concourse.bass_utils import run_bass_kernel_spmd

    nc = _get_nc()
    res = run_bass_kernel_spmd(
        nc, in_maps, core_ids=list(range(NCORES)), trace=trace
    )
    return res


def kernel(features, targets, cams, epoch, global_memory, all_pseudo_label,
           all_proxy_label, cam_proxies, label_proxies, _want_trace=False):
    import ml_dtypes

    feat = np.ascontiguousarray(np.asarray(features), dtype=np.float32)
    mem = np.ascontiguousarray(np.asarray(global_memory), dtype=np.float32)
    targets = np.asarray(targets).astype(np.int64)
    cams_h = np.asarray(cams).astype(np.int64)
    apl = np.asarray(all_proxy_label).astype(np.int64)
    apsl = np.asarray(all_pseudo_label).astype(np.int64)
    cam_prox = np.asarray(cam_proxies).astype(np.int64)   # [C, PSH]
    lab_prox = np.asarray(label_proxies).astype(np.int64)

    prx = apl[targets]                      # [B] target proxy
    pseudo_y = apsl[targets]                # [B]
    pos_cols = lab_prox[pseudo_y]           # [B, C] positive proxies (cross)
    memprx = mem[prx]                       # [B, D]

    # camera of each proxy; core k owns exactly camera k's proxies
    cam_of_p = np.empty(P, np.int64)
    cam_of_p[cam_prox.reshape(-1)] = np.repeat(np.arange(C), cam_prox.shape[1])
    assert cam_prox.shape == (C, PSH)

    memT = mem.T                            # [D, P]
    lhs2 = feat + np.float32(RATIO) * memprx           # [B, D] sims rows
    # Row-global exp bias: x = INV_TEMP * feat@mem_p with unit mem rows, so
    # x_row ~ N(0, (INV_TEMP*|feat|/sqrt(D))^2).  4.5 sigma sits within
    # [x_max - 80, x_max + ~25] for a 32768-sample max, so exp(x - mhat)
    # neither overflows nor flushes any term that matters.  Identical across
    # cores, so the merge is a plain sum.
    mhat = (4.5 * INV_TEMP / np.sqrt(D)) * np.linalg.norm(
        feat.astype(np.float64), axis=1)    # [B]
    nbias = np.ascontiguousarray((-mhat[:, None]).astype(np.float32))
    in_maps = []
    for k in range(NCORES):
        shard = memT[:, cam_prox[k]]                    # [D, 4096]
        packb = np.hstack([feat.T, shard])              # [D, 4352]
        pack8 = np.hstack([lhs2.T, shard])              # [D, 4352]
        in_maps.append({
            "packb": np.ascontiguousarray(packb.astype(ml_dtypes.bfloat16)),
            "pack8": np.ascontiguousarray(pack8.astype(ml_dtypes.float8_e4m3)),
            "nbias": nbias,
        })

    res = _run_device(in_maps, trace=_want_trace)
    results = res.results
    if _want_trace:
        _CACHE["last_exec_time_ns"] = res.exec_time_ns

    ob = np.stack([r["ob"] for r in results]).astype(np.float64)  # [K, B, OBW]
    zpart = ob[:, :, NWIN_CORE:]                                  # [K, B, NT]
    v8 = ob[:, :, :NWIN_CORE]                                     # [K, B, 64]

    rows = np.arange(B)

    # ---- logsumexp merge (cross / intra) ----
    mhat_used = -nbias[:, 0].astype(np.float64)               # exact bias device used
    Zc = zpart.sum(axis=2).T                                  # [B, C] (core k = cam k)
    lse_full = mhat_used + np.log(Zc.sum(axis=1))             # logsumexp over all P of x
    lse_cam = mhat_used + np.log(Zc[rows, cams_h])            # over own camera's proxies

    x_prx = INV_TEMP * np.einsum("bd,bd->b", feat.astype(np.float64),
                                 memprx.astype(np.float64))
    # If a sample's camera does not own its target proxy (possible when cams
    # is generated independently of targets), the reference's one-hot mask is
    # all-zero and its intra term is exactly 0.
    present = cam_of_p[prx] == cams_h
    intra = np.where(present, lse_cam - x_prx, 0.0)

    x_pos = INV_TEMP * np.einsum("bd,bkd->bk", feat.astype(np.float64),
                                 mem[pos_cols].astype(np.float64))
    cross = lse_full - x_pos.mean(axis=1)

    # ---- online loss ----
    # v8[k, b, w] = fp8-accurate max of sims' over window w of core/camera k
    # (proxies cam_prox[k, w*64 .. +64]).  Select candidate windows per row:
    # the global top windows (covers the reference's top-(BG_KNN+POSK)
    # proxies: the window holding the k-th largest value always ranks within
    # the top-k windows) plus every window within DELTA of its camera's best
    # (covers per-camera argmax).  Expand the selected windows and recompute
    # exact fp32 sims'/x there.  Margins sized for fp8 matmul noise
    # (sigma ~0.05 on window maxes).
    W = NCORES * NWIN_CORE                                    # 512 windows/row
    wv = np.moveaxis(v8, 0, 1).reshape(B, W)                  # [B, 512] k-major
    cam_of_w = np.repeat(np.arange(C), NWIN_CORE)             # [512]
    DELTA = 0.4
    JG = 88                                                   # global windows
    cammax = wv.reshape(B, C, NWIN_CORE).max(axis=2)          # [B, C]
    boost = wv >= (cammax[:, cam_of_w] - DELTA)               # near-camera-top
    nboost = int(boost.sum(axis=1).max())
    J = JG + max(nboost, C)
    prio = wv + 1e9 * boost
    sel_w = np.argpartition(-prio, J - 1, axis=1)[:, :J]      # [B, J] unique

    k_of = sel_w // NWIN_CORE                                 # camera/core
    w_of = sel_w % NWIN_CORE
    pid = cam_prox[k_of[:, :, None],
                   (w_of * WIN)[:, :, None] + np.arange(WIN)[None, None, :]]
    pid_b = pid.reshape(B, J * WIN)
    cam_of_cand = np.repeat(cam_of_w[sel_w], WIN, axis=1)     # [B, J*WIN]

    # exact fp32 recompute at the candidate proxies (row-chunked: the
    # gather is the memory hog)
    NCAND = J * WIN
    s_cand = np.empty((B, NCAND), np.float32)
    q_cand = np.empty((B, NCAND), np.float32)
    for lo in range(0, B, 32):
        hi = lo + 32
        memg = mem[pid_b[lo:hi]]                              # [32, NCAND, D]
        s_cand[lo:hi] = np.einsum("bd,bjd->bj", feat[lo:hi], memg)
        q_cand[lo:hi] = np.einsum("bd,bjd->bj", memprx[lo:hi], memg)
    simsp = s_cand.astype(np.float64) + RATIO * q_cand.astype(np.float64)
    x_cand = INV_TEMP * s_cand.astype(np.float64)

    # per-camera global argmax over candidates (exact values)
    tops_val = np.full((B, C), -np.inf)
    tops_j = np.zeros((B, C), np.int64)
    for c in range(C):
        sub = np.where(cam_of_cand == c, simsp, -np.inf)
        a = sub.argmax(axis=1)
        tops_j[:, c] = a
        tops_val[:, c] = sub[rows, a]

    # top-3 cameras by their best sims'
    order = np.argsort(-tops_val, axis=1)[:, :POSK]           # [B, 3]
    chosen_j = np.take_along_axis(tops_j, order, axis=1)      # [B, 3] cand idx
    chosen_pid = np.take_along_axis(pid_b, chosen_j, axis=1)  # [B, 3]

    # top-50 of the remaining candidates (windows are disjoint, so every
    # candidate proxy appears once; only the chosen need masking)
    is_chosen = (pid_b[:, :, None] == chosen_pid[:, None, :]).any(axis=2)
    Vmask = np.where(is_chosen, -np.inf, simsp)
    sel_idx = np.argpartition(-Vmask, BG_KNN, axis=1)[:, :BG_KNN]     # [B, 50]

    x_chosen = np.take_along_axis(x_cand, chosen_j, axis=1)   # [B, 3]
    x_sel = np.take_along_axis(x_cand, sel_idx, axis=1)       # [B, 50]
    xA = np.concatenate([x_chosen, x_sel], axis=1)            # [B, 53]
    mA = xA.max(axis=1)
    lse3 = mA + np.log(np.exp(xA - mA[:, None]).sum(axis=1))
    online = lse3 - x_chosen.mean(axis=1)

    # ---- camera-mean-sum ----
    dbg = globals().get("_DEBUG_COMPS")
    if dbg is not None:
        dbg["intra"] = intra.copy()
        dbg["cross"] = cross.copy()
        dbg["online"] = online.copy()
    total = 0.0
    for c in range(C):
        m = cams_h == c
        if m.any():
            total += intra[m].mean() + cross[m].mean() + online[m].mean()
    return np.float32(total)


# revision 10
# speedup vs baseline: 1.0036x; 1.0036x over previous
"""CameraAwareMemory loss kernel for 8 Trainium2 NeuronCores.

Strategy: camera-sharding — core k owns ALL 4096 proxies of camera k
(P=32768, C=8).  Each core computes score = feat @ memT with bf16
matmuls and sims' = (feat + r*mem[prx]) @ memT with fp8(e4m3) DoubleRow
matmuls (one instruction per 512-col half: the 256-deep contraction
rides the two packed k-halves), then reduces over 1024-col double-bank
PSUM tiles:
  - camera sum of exp(score/TEMP - mhat): one fused exp+accumulate on
    the scalar engine per 1024-col tile (4 partial sums per row-tile;
    host adds them).  mhat is a host-computed per-row bias, identical on
    all cores, statistically pinned to the row max.
  - per-64-proxy-window max of sims' via one DVE windowed tensor_reduce
    per 1024-col tile, direct on PSUM (16 windows per tile; the window
    POSITION identifies the proxies, so no max_index pass is needed)
The host merges the 8 cores' partials into the exact loss: the union of
the top-J windows per row provably covers every proxy the reference's
top-k selections can touch (a window containing the k-th largest value
always ranks within the top-k windows by window-max), and the host
recomputes exact fp32 scores at the candidate proxies so no selection
decision depends on fp8/bf16 rounding (fp8 only perturbs WHICH windows
are expanded; margins JG/DELTA absorb its ~0.05 value noise).
"""

import sys

import numpy as np

sys.path.insert(0, "/opt/trn_rl_repo")

# ---- problem constants (hardcoded per spec) ----
P = 32768
D = 256
C = 8
B = 256
TEMP = 0.05
BG_KNN = 50
POSK = 3
BAL_W = 0.15
RATIO = (1.0 - BAL_W) / BAL_W        # 5.666...: sims' = score + RATIO*q  (same order as sims)
INV_TEMP = 1.0 / TEMP                # 20.0
NCORES = 8
PSH = P // NCORES                    # 4096 proxies per core (= one camera)
WIN = 64                             # proxies per candidate window
NWIN_CORE = PSH // WIN               # 64 windows per core
# column subtiles per row-tile: two 512-wide leaders let the first
# activation start ~1.7us earlier (its operands arrive in one sliver)
TILES = [(0, 512), (512, 1024), (1024, 2048), (2048, 3072), (3072, 4096)]
NT = len(TILES)                      # 5
OBW = NWIN_CORE + NT                 # out cols per row-tile: 64 maxes + 5 partial sums

_CACHE = {}


def _build_bass():
    import concourse.bacc as bacc
    import concourse.mybir as mybir
    import concourse.tile as tile
    from contextlib import ExitStack

    f32 = mybir.dt.float32
    bf16 = mybir.dt.bfloat16
    f8 = mybir.dt.float8e4
    AF = mybir.ActivationFunctionType

    nc = bacc.Bacc("TRN2", target_bir_lowering=False, debug=False)

    # bf16 pack: [feat lhsT (256 cols) | memT shard (4096 cols)] per k-half
    PKB = 256 + PSH                       # 4352
    packb_d = nc.dram_tensor("packb", [D, PKB], bf16, kind="ExternalInput")
    # fp8 pack: [sims lhsT (256 cols) | memT shard (4096 cols)] per k-half
    pack8_d = nc.dram_tensor("pack8", [D, PKB], f8, kind="ExternalInput")
    nbias_d = nc.dram_tensor("nbias", [B, 1], f32, kind="ExternalInput")
    ob_d = nc.dram_tensor("ob", [B, OBW], f32, kind="ExternalOutput")

    with tile.TileContext(nc) as tc, ExitStack() as ctx:
        consts = ctx.enter_context(tc.tile_pool(name="consts", bufs=1))
        pqp = ctx.enter_context(tc.tile_pool(name="pq", bufs=2, space="PSUM"))
        psp = ctx.enter_context(tc.tile_pool(name="psc", bufs=2, space="PSUM"))
        small = ctx.enter_context(tc.tile_pool(name="small", bufs=2))
        outp = ctx.enter_context(tc.tile_pool(name="outp", bufs=2))

        packb_sb = consts.tile([128, 2 * PKB], bf16, tag="packb")
        pack8_sb = consts.tile([128, 2 * PKB], f8, tag="pack8")
        pb_src = packb_d.rearrange("(k p) c -> p k c", k=2)
        pb_dst = packb_sb.rearrange("p (k c) -> p k c", k=2)
        p8_src = pack8_d.rearrange("(k p) c -> p k c", k=2)
        p8_dst = pack8_sb.rearrange("p (k c) -> p k c", k=2)

        # DMA order = first-use order: fp8 leads bf16 for every column range
        # (the in-order PE queue issues the fp8 sims matmul of a tile before
        # its bf16 score matmuls).  The first tile's operands ship as small
        # slivers so its activation can start ~1.7us earlier.
        nc.sync.dma_start(out=p8_dst[:, :, 0:768], in_=p8_src[:, :, 0:768])
        nc.sync.dma_start(out=pb_dst[:, 0, 0:768], in_=pb_src[:, 0, 0:768])
        nc.sync.dma_start(out=pb_dst[:, 1, 0:768], in_=pb_src[:, 1, 0:768])
        nc.sync.dma_start(out=p8_dst[:, :, 768:1280], in_=p8_src[:, :, 768:1280])
        nc.sync.dma_start(out=pb_dst[:, :, 768:1280], in_=pb_src[:, :, 768:1280])
        bbounds = [1280, 2304, 3328, PKB]
        for g in range(len(bbounds) - 1):
            lo, hi = bbounds[g], bbounds[g + 1]
            nc.sync.dma_start(out=p8_dst[:, :, lo:hi], in_=p8_src[:, :, lo:hi])
            nc.sync.dma_start(out=pb_dst[:, :, lo:hi], in_=pb_src[:, :, lo:hi])

        # warm scratch init on the otherwise-idle gpsimd queue, BEFORE the
        # nbias DMAs so the PE warm-up isn't blocked behind them.  The warm
        # matmuls keep the PE busy from ~1us so the clock is ramped when the
        # real matmuls start; a dummy activation preloads the Exp table.
        warm_in = consts.tile([128, 640], bf16, tag="warm")
        warm_f = consts.tile([128, 8], f32, tag="warmf")
        nc.gpsimd.memset(warm_f[:], 0.0)
        nc.gpsimd.memset(warm_in[:], 0.0)
        nc.scalar.activation(warm_f[:], warm_f[:], AF.Exp)

        # per-row-tile -mhat bias (tiny; idle gpsimd queue, needed later)
        mbneg = []
        for rt in range(2):
            t = small.tile([128, 1], f32, tag=f"mbneg{rt}", name=f"mbneg_{rt}")
            nc.gpsimd.dma_start(out=t[:], in_=nbias_d[rt * 128:(rt + 1) * 128, :])
            mbneg.append(t)

        wl, wr = warm_in[:, 0:128], warm_in[:, 128:640]
        warm_ps = pqp.tile([128, 512], f32, tag="ps", name="warm_ps")
        for _ in range(4):
            nc.tensor.matmul(warm_ps[:], lhsT=wl, rhs=wr, start=True, stop=True)

        ob_t = []
        for rt in range(2):
            ob_t.append(outp.tile([128, OBW], f32, tag=f"ob{rt}", name=f"ob_{rt}"))

        packb_3d = packb_sb.rearrange("p (k c) -> p k c", k=2)
        pack8_3d = pack8_sb.rearrange("p (k c) -> p k c", k=2)

        def wb_ap(k, rt):                      # bf16 feat lhsT [128, 128]
            return packb_3d[:, k, rt * 128:(rt + 1) * 128]

        def w8_ap(rt):                         # fp8 sims lhsT [128, 2, 128]
            return pack8_3d[:, :, rt * 128:(rt + 1) * 128]

        def rhsb(k, lo, hi):                   # bf16 memT cols [128, w]
            return packb_3d[:, k, 256 + lo: 256 + hi]

        def rhs8(lo, hi):                      # fp8 memT cols [128, 2, w]
            return pack8_3d[:, :, 256 + lo: 256 + hi]

        DR = mybir.MatmulPerfMode.DoubleRow

        for t, (lo, hi) in enumerate(TILES):
            tw = hi - lo
            for rt in range(2):  # row-tile: batch rows [rt*128, rt*128+128)
                ps_q = pqp.tile([128, tw], f32, tag="ps", name=f"psq_{t}_{rt}")
                # fp8 DoubleRow: both k-halves in one pass, 512 cols per
                # instruction (fp8 moving-operand limit)
                for h in range(tw // 512):
                    nc.tensor.matmul(
                        ps_q[:, h * 512:(h + 1) * 512],
                        lhsT=w8_ap(rt),
                        rhs=rhs8(lo + h * 512, lo + (h + 1) * 512),
                        start=True, stop=True, perf_mode=DR,
                    )
                # window maxes of sims': one DVE scan over the whole subtile
                nc.vector.tensor_reduce(
                    out=ob_t[rt][:, lo // WIN: hi // WIN],
                    in_=ps_q[:].rearrange("p (w c) -> p w c", c=WIN),
                    axis=mybir.AxisListType.X,
                    op=mybir.AluOpType.max,
                )
                # score matmuls (bf16, 512-col PSUM-bank halves) + one fused
                # exp/accumulate over the whole subtile, in place (PSUM write
                # port beats SBUF; only the accum_out column is consumed)
                ps_s = psp.tile([128, tw], f32, tag="ps", name=f"pss_{t}_{rt}")
                for h in range(tw // 512):
                    for k in range(2):
                        nc.tensor.matmul(
                            ps_s[:, h * 512:(h + 1) * 512],
                            lhsT=wb_ap(k, rt),
                            rhs=rhsb(k, lo + h * 512, lo + (h + 1) * 512),
                            start=(k == 0), stop=(k == 1),
                        )
                nc.scalar.activation(
                    ps_s[:], ps_s[:], AF.Exp,
                    bias=mbneg[rt][:], scale=INV_TEMP,
                    accum_out=ob_t[rt][:, NWIN_CORE + t: NWIN_CORE + t + 1],
                )

        # v8 windows ship as soon as the DVE stream drains; the tiny zpart
        # columns (gated by the later-finishing act stream) go separately so
        # the final transfer is minimal
        for rt in range(2):
            r0, r1 = rt * 128, (rt + 1) * 128
            nc.sync.dma_start(out=ob_d[r0:r1, :NWIN_CORE],
                              in_=ob_t[rt][:, :NWIN_CORE])
        for rt in range(2):
            r0, r1 = rt * 128, (rt + 1) * 128
            nc.sync.dma_start(out=ob_d[r0:r1, NWIN_CORE:],
                              in_=ob_t[rt][:, NWIN_CORE:])

    nc.compile()
    return nc


def _get_nc():
    if "nc" not in _CACHE:
        _CACHE["nc"] = _build_bass()
    return _CACHE["nc"]


def _run_device(in_maps, trace=False):
    from concourse.bass_utils import run_bass_kernel_spmd

    nc = _get_nc()
    res = run_bass_kernel_spmd(
        nc, in_maps, core_ids=list(range(NCORES)), trace=trace
    )
    return res


def kernel(features, targets, cams, epoch, global_memory, all_pseudo_label,
           all_proxy_label, cam_proxies, label_proxies, _want_trace=False):
    import ml_dtypes

    feat = np.ascontiguousarray(np.asarray(features), dtype=np.float32)
    mem = np.ascontiguousarray(np.asarray(global_memory), dtype=np.float32)
    targets = np.asarray(targets).astype(np.int64)
    cams_h = np.asarray(cams).astype(np.int64)
    apl = np.asarray(all_proxy_label).astype(np.int64)
    apsl = np.asarray(all_pseudo_label).astype(np.int64)
    cam_prox = np.asarray(cam_proxies).astype(np.int64)   # [C, PSH]
    lab_prox = np.asarray(label_proxies).astype(np.int64)

    prx = apl[targets]                      # [B] target proxy
    pseudo_y = apsl[targets]                # [B]
    pos_cols = lab_prox[pseudo_y]           # [B, C] positive proxies (cross)
    memprx = mem[prx]                       # [B, D]

    # camera of each proxy; core k owns exactly camera k's proxies
    cam_of_p = np.empty(P, np.int64)
    cam_of_p[cam_prox.reshape(-1)] = np.repeat(np.arange(C), cam_prox.shape[1])
    assert cam_prox.shape == (C, PSH)

    memT = mem.T                            # [D, P]
    lhs2 = feat + np.float32(RATIO) * memprx           # [B, D] sims rows
    # Row-global exp bias: x = INV_TEMP * feat@mem_p with unit mem rows, so
    # x_row ~ N(0, (INV_TEMP*|feat|/sqrt(D))^2).  4.5 sigma sits within
    # [x_max - 80, x_max + ~25] for a 32768-sample max, so exp(x - mhat)
    # neither overflows nor flushes any term that matters.  Identical across
    # cores, so the merge is a plain sum.
    mhat = (4.5 * INV_TEMP / np.sqrt(D)) * np.linalg.norm(
        feat.astype(np.float64), axis=1)    # [B]
    nbias = np.ascontiguousarray((-mhat[:, None]).astype(np.float32))
    in_maps = []
    for k in range(NCORES):
        shard = memT[:, cam_prox[k]]                    # [D, 4096]
        packb = np.hstack([feat.T, shard])              # [D, 4352]
        pack8 = np.hstack([lhs2.T, shard])              # [D, 4352]
        in_maps.append({
            "packb": np.ascontiguousarray(packb.astype(ml_dtypes.bfloat16)),
            "pack8": np.ascontiguousarray(pack8.astype(ml_dtypes.float8_e4m3)),
            "nbias": nbias,
        })

    res = _run_device(in_maps, trace=_want_trace)
    results = res.results
    if _want_trace:
        _CACHE["last_exec_time_ns"] = res.exec_time_ns

    ob = np.stack([r["ob"] for r in results]).astype(np.float64)  # [K, B, OBW]
    zpart = ob[:, :, NWIN_CORE:]                                  # [K, B, NT]
    v8 = ob[:, :, :NWIN_CORE]                                     # [K, B, 64]

    rows = np.arange(B)

    # ---- logsumexp merge (cross / intra) ----
    mhat_used = -nbias[:, 0].astype(np.float64)               # exact bias device used
    Zc = zpart.sum(axis=2).T                                  # [B, C] (core k = cam k)
    lse_full = mhat_used + np.log(Zc.sum(axis=1))             # logsumexp over all P of x
    lse_cam = mhat_used + np.log(Zc[rows, cams_h])            # over own camera's proxies

    x_prx = INV_TEMP * np.einsum("bd,bd->b", feat.astype(np.float64),
                                 memprx.astype(np.float64))
    # If a sample's camera does not own its target proxy (possible when cams
    # is generated independently of targets), the reference's one-hot mask is
    # all-zero and its intra term is exactly 0.
    present = cam_of_p[prx] == cams_h
    intra = np.where(present, lse_cam - x_prx, 0.0)

    x_pos = INV_TEMP * np.einsum("bd,bkd->bk", feat.astype(np.float64),
                                 mem[pos_cols].astype(np.float64))
    cross = lse_full - x_pos.mean(axis=1)

    # ---- online loss ----
    # v8[k, b, w] = fp8-accurate max of sims' over window w of core/camera k
    # (proxies cam_prox[k, w*64 .. +64]).  Select candidate windows per row:
    # the global top windows (covers the reference's top-(BG_KNN+POSK)
    # proxies: the window holding the k-th largest value always ranks within
    # the top-k windows) plus every window within DELTA of its camera's best
    # (covers per-camera argmax).  Expand the selected windows and recompute
    # exact fp32 sims'/x there.  Margins sized for fp8 matmul noise
    # (sigma ~0.05 on window maxes).
    W = NCORES * NWIN_CORE                                    # 512 windows/row
    wv = np.moveaxis(v8, 0, 1).reshape(B, W)                  # [B, 512] k-major
    cam_of_w = np.repeat(np.arange(C), NWIN_CORE)             # [512]
    DELTA = 0.4
    JG = 88                                                   # global windows
    cammax = wv.reshape(B, C, NWIN_CORE).max(axis=2)          # [B, C]
    boost = wv >= (cammax[:, cam_of_w] - DELTA)               # near-camera-top
    nboost = int(boost.sum(axis=1).max())
    J = JG + max(nboost, C)
    prio = wv + 1e9 * boost
    sel_w = np.argpartition(-prio, J - 1, axis=1)[:, :J]      # [B, J] unique

    k_of = sel_w // NWIN_CORE                                 # camera/core
    w_of = sel_w % NWIN_CORE
    pid = cam_prox[k_of[:, :, None],
                   (w_of * WIN)[:, :, None] + np.arange(WIN)[None, None, :]]
    pid_b = pid.reshape(B, J * WIN)
    cam_of_cand = np.repeat(cam_of_w[sel_w], WIN, axis=1)     # [B, J*WIN]

    # exact fp32 recompute at the candidate proxies (row-chunked: the
    # gather is the memory hog)
    NCAND = J * WIN
    s_cand = np.empty((B, NCAND), np.float32)
    q_cand = np.empty((B, NCAND), np.float32)
    for lo in range(0, B, 32):
        hi = lo + 32
        memg = mem[pid_b[lo:hi]]                              # [32, NCAND, D]
        s_cand[lo:hi] = np.einsum("bd,bjd->bj", feat[lo:hi], memg)
        q_cand[lo:hi] = np.einsum("bd,bjd->bj", memprx[lo:hi], memg)
    simsp = s_cand.astype(np.float64) + RATIO * q_cand.astype(np.float64)
    x_cand = INV_TEMP * s_cand.astype(np.float64)

    # per-camera global argmax over candidates (exact values)
    tops_val = np.full((B, C), -np.inf)
    tops_j = np.zeros((B, C), np.int64)
    for c in range(C):
        sub = np.where(cam_of_cand == c, simsp, -np.inf)
        a = sub.argmax(axis=1)
        tops_j[:, c] = a
        tops_val[:, c] = sub[rows, a]

    # top-3 cameras by their best sims'
    order = np.argsort(-tops_val, axis=1)[:, :POSK]           # [B, 3]
    chosen_j = np.take_along_axis(tops_j, order, axis=1)      # [B, 3] cand idx
    chosen_pid = np.take_along_axis(pid_b, chosen_j, axis=1)  # [B, 3]

    # top-50 of the remaining candidates (windows are disjoint, so every
    # candidate proxy appears once; only the chosen need masking)
    is_chosen = (pid_b[:, :, None] == chosen_pid[:, None, :]).any(axis=2)
    Vmask = np.where(is_chosen, -np.inf, simsp)
    sel_idx = np.argpartition(-Vmask, BG_KNN, axis=1)[:, :BG_KNN]     # [B, 50]

    x_chosen = np.take_along_axis(x_cand, chosen_j, axis=1)   # [B, 3]
    x_sel = np.take_along_axis(x_cand, sel_idx, axis=1)       # [B, 50]
    xA = np.concatenate([x_chosen, x_sel], axis=1)            # [B, 53]
    mA = xA.max(axis=1)
    lse3 = mA + np.log(np.exp(xA - mA[:, None]).sum(axis=1))
    online = lse3 - x_chosen.mean(axis=1)

    # ---- camera-mean-sum ----
    dbg = globals().get("_DEBUG_COMPS")
    if dbg is not None:
        dbg["intra"] = intra.copy()
        dbg["cross"] = cross.copy()
        dbg["online"] = online.copy()
    total = 0.0
    for c in range(C):
        m = cams_h == c
        if m.any():
            total += intra[m].mean() + cross[m].mean() + online[m].mean()
    return np.float32(total)
